# revision 19
# baseline (speedup 1.0000x reference)
"""Trainium2 Bass kernel for LoRA self-attention (nn_LoRAAttnProcessor).

Problem shapes (hardcoded): x [2, 2048, 1280], 20 heads x 64 dim, LoRA rank 4.

Strategy
--------
* Host side: fold every LoRA pair into its base weight (W_eff = W + B @ A) --
  mathematically identical (associativity), and fold the 1/sqrt(D) score
  scale into Wq_eff.  The kernel then computes plain multi-head attention.
* Sharding: 8 cores x (batch b = core//4, 5 heads = core%4).  Wq/Wk/Wv are
  column-sharded by head, Wo row-sharded by head; each core emits a partial
  TRANSPOSED output [1280, 2048] that the host sums per batch element and
  transposes (+ bias bo).
* Per core (S=2048, C=1280, 5 local heads, D=64), all matmuls in bf16 with
  fp32 PSUM accumulation.  Schedule: chunked input DMAs (host pre-shuffles
  weights/activations into partition-major blocks so every DMA moves large
  contiguous runs), software pipelining of the projection work (A1 qk-proj,
  A2 v-proj, transposed out-proj) into the attention phase, dedicated PSUM
  pools (scores x2 | ctx | bcast | aux = 8 banks exactly), matmul-based
  softmax-denominator broadcast, and output DMAs issued from the otherwise
  idle GPSIMD queue so they never serialize against next-iteration input
  DMAs on the SP queue.
"""

import os
import sys

if "/opt/trn_rl_repo" not in sys.path:
    sys.path.insert(0, "/opt/trn_rl_repo")

from contextlib import ExitStack

import ml_dtypes
import numpy as np

import concourse.bass as bass
import concourse.tile as tile
from concourse import bacc, mybir
from concourse.bass_utils import run_bass_kernel_spmd

BF16 = mybir.dt.bfloat16
F32 = mybir.dt.float32
NPBF16 = ml_dtypes.bfloat16

D = 64
H_LOC = 5  # heads per core
N_CORES = 8


def _q_loc(h):
    """(chunk, partition offset) of qT for local head h in qkT_sb."""
    return (h // 2, (h % 2) * 64) if h < 4 else (4, 0)


def _k_loc(h):
    return (2 + h // 2, (h % 2) * 64) if h < 4 else (5, 0)


FLIP = os.environ.get("KFLIP", "1") == "1"


def build_program(S=2048, C=1280, paired=False, interleave=False, repeat=1,
                  flip=None):
    """Build the SPMD single-core program. S == 2048, C == 1280.

    flip=False: ctx accumulated k-major as ctxT [65, q] (2 fat matmuls/step),
    softmax denominator broadcast via ones-matmul.
    flip=True: ctx accumulated q-major as [q, 65] (8 thin matmuls/step, half
    the PE stream cycles), per-partition normalize, PE-transpose back to ctxT.
    """
    if flip is None:
        flip = FLIP
    assert S == 2048 and C % 128 == 0
    CK = C // 128         # contraction chunks over channels (10)
    SN = S // 512         # 512-col slices of sequence (4)
    SQB = 1024            # query block width
    NSQ = S // SQB        # 2
    SK = S // 128         # key chunks (16)
    NQ = SQB // 512       # 2
    CCH = C // 128        # out-proj column chunks (10)

    nc = bacc.Bacc("TRN2", target_bir_lowering=False, debug=False)

    # Host pre-shuffled, partition-major inputs (big contiguous DMA runs).
    xT_d = nc.dram_tensor("xT", [128, SN, CK, 512], BF16, kind="ExternalInput").ap()
    wqk_d = nc.dram_tensor("wqk", [128, 6, CK, 128], BF16, kind="ExternalInput").ap()
    wvT_d = nc.dram_tensor(
        "wvT", [128, CK, H_LOC * D], BF16, kind="ExternalInput"
    ).ap()
    woT_d = nc.dram_tensor("woT", [128, 3, C], BF16, kind="ExternalInput").ap()
    if flip:
        ident_d = nc.dram_tensor("ident", [128, 128], BF16, kind="ExternalInput").ap()
    out_d = nc.dram_tensor("outT_part", [C, S], F32, kind="ExternalOutput").ap()

    EXP = mybir.ActivationFunctionType.Exp
    MULT = mybir.AluOpType.mult

    with tile.TileContext(nc) as tc, ExitStack() as ctx:
        persist = ctx.enter_context(tc.tile_pool(name="persist", bufs=1))
        # PSUM budget (8 banks of 2KB):
        #   spool 2 x [128,1024]f32 = 4 | cpool [*,1024]f32 = 2
        #   bcpool/psT = 1             | aux [128,512]f32 = 1
        spool = ctx.enter_context(tc.tile_pool(name="sc", bufs=2, space="PSUM"))
        cpool = ctx.enter_context(tc.tile_pool(name="cx", bufs=1, space="PSUM"))
        bcpool = ctx.enter_context(tc.tile_pool(name="bc", bufs=1, space="PSUM"))
        aux = ctx.enter_context(tc.tile_pool(name="aux", bufs=1, space="PSUM"))
        ppool = ctx.enter_context(tc.tile_pool(name="probs", bufs=4))
        smallp = ctx.enter_context(tc.tile_pool(name="small", bufs=2))
        outp = ctx.enter_context(tc.tile_pool(name="osb", bufs=8))

        xT_sb = persist.tile([128, SN, CK, 512], BF16, tag="xT")
        wqk_sb = persist.tile([128, 6, CK, 128], BF16, tag="wqk")
        wvT_sb = persist.tile([128, CK, H_LOC * D], BF16, tag="wvT")
        woT_sb = persist.tile([128, 3, C], BF16, tag="woT")
        qkT_sb = persist.tile([128, 6, S], BF16, tag="qkT")
        v_sb = persist.tile([128, SK, H_LOC, D + 1], BF16, tag="vsb")
        ctxT_sb = persist.tile([128, 3, S], BF16, tag="ctxT")
        ones_sb = persist.tile([1, D], BF16, tag="ones")
        if flip:
            ident_sb = persist.tile([128, 128], BF16, tag="ident")

        def emit_body(rep):
            # ---- chunked input DMAs, in consumption order ----
            def dma_wqk(f):
                nc.sync.dma_start(wqk_sb[:, f], wqk_d[:, f])

            def dma_xt(s):
                nc.sync.dma_start(xT_sb[:, s], xT_d[:, s])

            dma_wqk(0)
            dma_xt(0)
            dma_wqk(2)
            nc.sync.dma_start(wvT_sb[:], wvT_d[:])
            dma_xt(1)
            dma_wqk(1)
            dma_wqk(3)
            dma_xt(2)
            dma_xt(3)
            dma_wqk(4)
            dma_wqk(5)
            nc.sync.dma_start(woT_sb[:], woT_d[:])
            if flip:
                nc.sync.dma_start(ident_sb[:], ident_d[:])

            # ones column for the softmax-denominator trick; zero the 64 pad
            # partitions of the last ctxT chunk (head 4 has no pair); ones row
            # for the denominator-broadcast matmul.
            nc.vector.memset(v_sb[:, :, :, D : D + 1], 1.0)
            nc.vector.memset(ctxT_sb[64:128, 2, :], 0.0)
            nc.vector.memset(ones_sb[:], 1.0)

            # ---- unit emitters ----
            def a1(f, s, pool):
                """qkT chunk: [128 feat, 512 seq] = wqk_f^T @ xT_slice."""
                ps = pool.tile([128, 512], F32, tag="sc" if pool is spool else "aux",
                               name=f"a1ps_{rep}_{f}_{s}")
                for c in range(CK):
                    nc.tensor.matmul(
                        ps[:, 0:512],
                        lhsT=wqk_sb[:, f, c, :],
                        rhs=xT_sb[:, s, c, :],
                        start=(c == 0),
                        stop=(c == CK - 1),
                    )
                nc.vector.tensor_copy(
                    out=qkT_sb[:, f, s * 512 : (s + 1) * 512], in_=ps[:, 0:512]
                )

            def a2(m, pool):
                """v chunk: [128 seq, 320 feat] = xT_m^T @ wvT."""
                ps = pool.tile([128, 512], F32, tag="sc" if pool is spool else "aux",
                               name=f"a2ps_{rep}_{m}")
                s, j = m // 4, (m % 4) * 128
                for c in range(CK):
                    nc.tensor.matmul(
                        ps[:, 0 : H_LOC * D],
                        lhsT=xT_sb[:, s, c, j : j + 128],
                        rhs=wvT_sb[:, c, :],
                        start=(c == 0),
                        stop=(c == CK - 1),
                    )
                nc.vector.tensor_copy(
                    out=v_sb[:, m, :, 0:D],
                    in_=ps[:, 0 : H_LOC * D].rearrange("p (h d) -> p h d", h=H_LOC),
                )

            def op(cc, qb, pool, ptag):
                """outT block: [128 cols, 512 seq] = Wo_loc^T-chunk @ ctxT."""
                ps = pool.tile([128, 512], F32, tag=ptag, name=f"ops_{rep}_{cc}_{qb}")
                for j in range(3):
                    nc.tensor.matmul(
                        ps[:, 0:512],
                        lhsT=woT_sb[:, j, cc * 128 : (cc + 1) * 128],
                        rhs=ctxT_sb[:, j, qb * 512 : (qb + 1) * 512],
                        start=(j == 0),
                        stop=(j == 2),
                    )
                ob = outp.tile([128, 512], F32, tag="osb", name=f"ob_{rep}_{cc}_{qb}")
                nc.vector.tensor_copy(out=ob[:], in_=ps[:, 0:512])
                nc.gpsimd.dma_start(
                    out_d[cc * 128 : (cc + 1) * 128, qb * 512 : (qb + 1) * 512], ob[:]
                )

            # ---- attention units ----
            def scores_unit(h, sq, sk):
                """scoresT psum [128 k, 1024 q] + exp -> probs bf16."""
                qc, qo = _q_loc(h)
                kc, ko = _k_loc(h)
                sc = spool.tile([128, SQB], F32, tag="sc", name=f"sc_{h}_{sq}_{sk}")
                for n in range(NQ):
                    nc.tensor.matmul(
                        sc[:, n * 512 : (n + 1) * 512],
                        lhsT=qkT_sb[ko : ko + D, kc, sk * 128 : (sk + 1) * 128],
                        rhs=qkT_sb[
                            qo : qo + D,
                            qc,
                            sq * SQB + n * 512 : sq * SQB + (n + 1) * 512,
                        ],
                        start=True,
                        stop=True,
                    )
                pt = ppool.tile([128, SQB], BF16, tag="probs", name=f"pt_{h}_{sq}_{sk}")
                nc.scalar.activation(pt[:, 0:SQB], sc[:, 0:SQB], EXP)
                return pt

            def ctx_unit(h, sq, sk, pt, ctx_ps):
                if flip:
                    # PSUM start zeroes a whole 2KB bank (lazy pending-zero):
                    # only the first qc group of each bank may issue start, the
                    # other three inherit the bank's pending-zero state.
                    for qc in range(SQB // 128):
                        nc.tensor.matmul(
                            ctx_ps[:, qc, 0 : D + 1],
                            lhsT=pt[:, qc * 128 : (qc + 1) * 128],
                            rhs=v_sb[:, sk, h, :],
                            start=(sk == 0 and qc % 4 == 0),
                            stop=(sk == SK - 1 and qc % 4 == 0),
                            skip_group_check=True,
                        )
                else:
                    for n in range(NQ):
                        nc.tensor.matmul(
                            ctx_ps[0 : D + 1, n * 512 : (n + 1) * 512],
                            lhsT=v_sb[:, sk, h, :],
                            rhs=pt[:, n * 512 : (n + 1) * 512],
                            start=(sk == 0),
                            stop=(sk == SK - 1),
                        )

            def nrm(h, sq, ctx_ps):
                """Normalize + store ctxT bf16 for the out-projection."""
                jc, po = h // 2, (h % 2) * 64
                if flip:
                    # per-partition softmax normalize, then PE-transpose back
                    rec = smallp.tile([128, 8], F32, tag="recf", name=f"rec_{h}_{sq}")
                    nc.vector.reciprocal(rec[:], ctx_ps[:, :, D])
                    ctxn = smallp.tile(
                        [128, 8, D], BF16, tag="ctxn", name=f"ctxn_{h}_{sq}"
                    )
                    for qc in range(8):
                        nc.vector.tensor_scalar_mul(
                            ctxn[:, qc, :], ctx_ps[:, qc, 0:D], rec[:, qc : qc + 1]
                        )
                    psT = _flip_state.get("psT")
                    if psT is None:
                        psT = bcpool.tile(
                            [128, SQB], BF16, tag="bc", name=f"psT_{h}_{sq}"
                        )
                        _flip_state["psT"] = psT
                    for qc in range(8):
                        nc.tensor.transpose(
                            psT[po : po + D, qc * 128 : (qc + 1) * 128],
                            ctxn[:, qc, :],
                            ident_sb[:],
                        )
                    if h % 2 == 1 or h == 4:
                        rows = 128 if h % 2 == 1 else 64
                        nc.vector.tensor_copy(
                            out=ctxT_sb[0:rows, jc, sq * SQB : (sq + 1) * SQB],
                            in_=psT[0:rows, :],
                        )
                        _flip_state["psT"] = None
                else:
                    rec = smallp.tile([1, SQB], BF16, tag="rec", name=f"rec_{h}_{sq}")
                    with nc.allow_low_precision(reason="softmax recip bf16"):
                        nc.vector.reciprocal(rec[:], ctx_ps[D : D + 1, 0:SQB])
                    for half in range(2):
                        c0 = half * 512
                        bc = bcpool.tile(
                            [D, 512], F32, tag="bc", name=f"bc_{h}_{sq}_{half}"
                        )
                        nc.tensor.matmul(
                            bc[:, 0:512],
                            lhsT=ones_sb[0:1, :],
                            rhs=rec[0:1, c0 : c0 + 512],
                            start=True,
                            stop=True,
                        )
                        # DVE cannot read two PSUM operands; stage bc in SBUF
                        bcs = smallp.tile(
                            [D, 512], BF16, tag="bcs", name=f"bcs_{h}_{sq}_{half}"
                        )
                        nc.vector.tensor_copy(out=bcs[:], in_=bc[:, 0:512])
                        nc.vector.tensor_tensor(
                            out=ctxT_sb[
                                po : po + D, jc, sq * SQB + c0 : sq * SQB + c0 + 512
                            ],
                            in0=ctx_ps[0:D, c0 : c0 + 512],
                            in1=bcs[:],
                            op=MULT,
                        )

            _flip_state = {"psT": None}

            # ---- lead-in: enough projection work to start attention ----
            a1(0, 0, spool)
            a1(0, 1, spool)
            a1(2, 0, spool)
            a2(0, spool)
            a2(1, spool)

            # ---- filler schedule: (sq, h) -> 16 per-iteration buckets.
            # Ordering constraints: scores(h, sq, sk) needs k-chunk slice
            # floor(sk/4) of the head's k block BEFORE iteration sk, and
            # ctx(h, sq, m) (emitted at iteration m+1, before fillers) needs
            # a2(m) emitted at iteration <= m.
            def A1(f, s):
                return lambda: a1(f, s, aux)

            def A2(m):
                return lambda: a2(m, aux)

            def OP(c, q):
                return lambda: op(c, q, aux, "aux")

            def spread(fillers, step=2, start=0):
                buckets = [[] for _ in range(SK)]
                for i, f in enumerate(fillers):
                    buckets[min(start + i * step, SK - 1)].append(f)
                return buckets

            fill = {
                (0, 0): [
                    [A1(2, 1), A2(2)], [A2(3)], [A2(4)],
                    [A1(2, 2), A2(5)], [A2(6)], [A2(7)],
                    [A1(2, 3), A2(8)], [A2(9)], [A2(10)], [A2(11)], [A2(12)],
                    [A2(13)], [A2(14)], [A2(15)], [], [],
                ],
                (0, 1): spread(
                    [A1(1, 0), A1(3, 0), A1(1, 1), A1(3, 1), A1(3, 2), A1(3, 3)]
                ),
                (0, 2): spread(
                    [A1(4, 0), A1(5, 0), A1(4, 1), A1(5, 1), A1(5, 2), A1(5, 3)]
                ),
                (0, 3): spread([A1(0, 2), A1(0, 3), A1(1, 2), A1(1, 3)]),
                (0, 4): spread([A1(4, 2), A1(4, 3)]),
                # start=2: sq0's ctxT is only complete after the deferred
                # nrm of (0,4), which is emitted at iteration sk==1 here
                (1, 0): spread(
                    [OP(c, q) for c in range(4) for q in range(2)], start=2
                ),
                (1, 1): spread([OP(c, q) for c in range(4, 8) for q in range(2)]),
                (1, 2): spread([OP(c, q) for c in range(8, 10) for q in range(2)]),
                (1, 3): [[] for _ in range(SK)],
                (1, 4): [[] for _ in range(SK)],
            }

            # ---- main attention stream (sq-outer), software-pipelined ----
            prev_nrm = None  # deferred normalize of the previous head
            ctx_shape = [128, 8, 128] if flip else [128, SQB]
            for sq in range(NSQ):
                for h in range(H_LOC):
                    buckets = fill[(sq, h)]
                    # ctx tile is allocated lazily AFTER the previous head's
                    # deferred normalize is emitted: cpool has bufs=1, so the
                    # slot must have all its readers emitted before reuse.
                    ctx_ps = None
                    pts = {}
                    for sk in range(SK):
                        pts[sk] = scores_unit(h, sq, sk)
                        if sk == 1 and prev_nrm is not None:
                            # normalize the previous head once this head's
                            # first scores are in flight (frees its ctx tile
                            # well before our first ctx matmul needs it)
                            prev_nrm()
                            prev_nrm = None
                        if sk > 0:
                            if ctx_ps is None:
                                ctx_ps = cpool.tile(
                                    ctx_shape, F32, tag="ctx", name=f"ctx_{h}_{sq}"
                                )
                            ctx_unit(h, sq, sk - 1, pts.pop(sk - 1), ctx_ps)
                        for f in buckets[sk]:
                            f()
                    ctx_unit(h, sq, SK - 1, pts.pop(SK - 1), ctx_ps)
                    prev_nrm = (lambda hh, ss, cp: lambda: nrm(hh, ss, cp))(
                        h, sq, ctx_ps
                    )
            prev_nrm()

            # ---- tail: remaining out-proj blocks, rotating over freed pools ----
            pools = [(spool, "sc"), (cpool, "ctx"), (aux, "aux"), (spool, "sc")]
            i = 0
            for cc in range(CCH):
                for qb in (2, 3):
                    p, t = pools[i % len(pools)]
                    op(cc, qb, p, t)
                    i += 1

        for rep in range(repeat):
            emit_body(rep)

    nc.compile()
    return nc


def make_core_inputs(x, Wq_eff, Wk_eff, Wv_eff, Wo_eff):
    """Per-core input dicts. x [B,S,C] f32; W_eff [C,C] f32 (scale folded).

    All tensors are pre-shuffled into the kernel's partition-major DMA
    layouts so every chunk DMA moves large contiguous runs.
    """
    B, S, C = x.shape
    CK = C // 128
    in_maps = []
    xT_p = []
    for b in range(B):
        xT = np.ascontiguousarray(x[b].T).astype(NPBF16)  # [C, S]
        xT_p.append(
            np.ascontiguousarray(
                xT.reshape(CK, 128, 4, 512).transpose(1, 2, 0, 3)
            )
        )  # [128, 4, CK, 512]
    ident = np.eye(128, dtype=np.float32).astype(NPBF16)
    for core in range(N_CORES):
        b, g = core // 4, core % 4
        r0 = g * H_LOC * D  # first feature row of this core's heads
        qf = Wq_eff[r0 : r0 + H_LOC * D]  # (320, C)
        kf = Wk_eff[r0 : r0 + H_LOC * D]
        vf = Wv_eff[r0 : r0 + H_LOC * D]
        zero = np.zeros((D, C), np.float32)
        # chunks: (q0,q1)(q2,q3)(k0,k1)(k2,k3)(q4,0)(k4,0)
        wqk = np.concatenate(
            [qf[: 4 * D], kf[: 4 * D], qf[4 * D :], zero, kf[4 * D :], zero], axis=0
        ).T  # (C, 768)
        wqk_p = (
            wqk.astype(NPBF16).reshape(CK, 128, 6, 128).transpose(1, 2, 0, 3)
        )  # [128, 6, CK, 128]
        wvT_p = (
            vf.T.astype(NPBF16).reshape(CK, 128, H_LOC * D).transpose(1, 0, 2)
        )  # [128, CK, 320]
        woT = np.concatenate(
            [Wo_eff[:, r0 : r0 + H_LOC * D].T, np.zeros((D, C), np.float32)], axis=0
        )  # (384, C)
        woT_p = woT.astype(NPBF16).reshape(3, 128, C).transpose(1, 0, 2)  # [128,3,C]
        in_maps.append(
            {
                "xT": xT_p[b],
                "wqk": np.ascontiguousarray(wqk_p),
                "wvT": np.ascontiguousarray(wvT_p),
                "woT": np.ascontiguousarray(woT_p),
                "ident": ident,
            }
        )
    return in_maps


def fold_weights(Wq, Wk, Wv, Wo, Aq, Bq, Ak, Bk, Av, Bv, Ao, Bo):
    scale = 1.0 / np.sqrt(np.float32(D))
    Wq_eff = (Wq + Bq @ Aq) * scale
    Wk_eff = Wk + Bk @ Ak
    Wv_eff = Wv + Bv @ Av
    Wo_eff = Wo + Bo @ Ao
    return Wq_eff, Wk_eff, Wv_eff, Wo_eff


_NC_CACHE = {}


def _get_program(S, C):
    key = (S, C)
    if key not in _NC_CACHE:
        _NC_CACHE[key] = build_program(S, C)
    return _NC_CACHE[key]


def kernel(**inputs):
    inputs = {k: np.asarray(v, np.float32) for k, v in inputs.items()}
    x = inputs["x"]
    B, S, C = x.shape
    Wq_eff, Wk_eff, Wv_eff, Wo_eff = fold_weights(
        inputs["Wq"], inputs["Wk"], inputs["Wv"], inputs["Wo"],
        inputs["Aq"], inputs["Bq"], inputs["Ak"], inputs["Bk"],
        inputs["Av"], inputs["Bv"], inputs["Ao"], inputs["Bo"],
    )
    in_maps = make_core_inputs(x, Wq_eff, Wk_eff, Wv_eff, Wo_eff)
    nc = _get_program(S, C)
    if not FLIP:
        for m in in_maps:
            m.pop("ident", None)  # non-flip program has no ident input
    res = run_bass_kernel_spmd(nc, in_maps, list(range(N_CORES)))
    parts = [res.results[c]["outT_part"].astype(np.float32) for c in range(N_CORES)]
    bo = inputs["bo"]
    out = np.stack(
        [
            (parts[0] + parts[1] + parts[2] + parts[3]).T + bo,
            (parts[4] + parts[5] + parts[6] + parts[7]).T + bo,
        ]
    ).astype(np.float32)
    return out


# revision 23
# speedup vs baseline: 1.0422x; 1.0422x over previous
"""Trainium2 Bass kernel for LoRA self-attention (nn_LoRAAttnProcessor).

Problem shapes (hardcoded): x [2, 2048, 1280], 20 heads x 64 dim, LoRA rank 4.

Strategy
--------
* Host side: fold every LoRA pair into its base weight (W_eff = W + B @ A) --
  mathematically identical (associativity), and fold the 1/sqrt(D) score
  scale into Wq_eff.  The kernel then computes plain multi-head attention.
* Sharding: 8 cores x (batch b = core//4, 5 heads = core%4).  Wq/Wk/Wv are
  column-sharded by head, Wo row-sharded by head; each core emits a partial
  TRANSPOSED output [1280, 2048] that the host sums per batch element and
  transposes (+ bias bo).
* Per core (S=2048, C=1280, 5 local heads, D=64), all matmuls in bf16 with
  fp32 PSUM accumulation.  Schedule: chunked input DMAs (host pre-shuffles
  weights/activations into partition-major blocks so every DMA moves large
  contiguous runs), software pipelining of the projection work (A1 qk-proj,
  A2 v-proj, transposed out-proj) into the attention phase, dedicated PSUM
  pools (scores x2 | ctx | bcast | aux = 8 banks exactly), matmul-based
  softmax-denominator broadcast, and output DMAs issued from the otherwise
  idle GPSIMD queue so they never serialize against next-iteration input
  DMAs on the SP queue.
"""

import os
import sys

if "/opt/trn_rl_repo" not in sys.path:
    sys.path.insert(0, "/opt/trn_rl_repo")

from contextlib import ExitStack

import ml_dtypes
import numpy as np

import concourse.bass as bass
import concourse.tile as tile
from concourse import bacc, mybir
from concourse.bass_utils import run_bass_kernel_spmd

BF16 = mybir.dt.bfloat16
F32 = mybir.dt.float32
NPBF16 = ml_dtypes.bfloat16

D = 64
H_LOC = 5  # heads per core
N_CORES = 8


def _q_loc(h):
    """(chunk, partition offset) of qT for local head h in qkT_sb."""
    return (h // 2, (h % 2) * 64) if h < 4 else (4, 0)


def _k_loc(h):
    return (2 + h // 2, (h % 2) * 64) if h < 4 else (5, 0)


# flip=True sims faster but measures ~30% slower on real HW: the 8 thin
# (N=65) ctx matmuls per step each reload 128 stationary rows, which the
# cost model treats as free but hardware cannot hide behind a 65-cycle
# stream. Keep the fat k-major ctx matmuls.
FLIP = os.environ.get("KFLIP", "0") == "1"


def build_program(S=2048, C=1280, paired=False, interleave=False, repeat=1,
                  flip=None):
    """Build the SPMD single-core program. S == 2048, C == 1280.

    flip=False: ctx accumulated k-major as ctxT [65, q] (2 fat matmuls/step),
    softmax denominator broadcast via ones-matmul.
    flip=True: ctx accumulated q-major as [q, 65] (8 thin matmuls/step, half
    the PE stream cycles), per-partition normalize, PE-transpose back to ctxT.
    """
    if flip is None:
        flip = FLIP
    assert S == 2048 and C % 128 == 0
    CK = C // 128         # contraction chunks over channels (10)
    SN = S // 512         # 512-col slices of sequence (4)
    SQB = 1024            # query block width
    NSQ = S // SQB        # 2
    SK = S // 128         # key chunks (16)
    NQ = SQB // 512       # 2
    CCH = C // 128        # out-proj column chunks (10)

    nc = bacc.Bacc("TRN2", target_bir_lowering=False, debug=False)

    # Host pre-shuffled, partition-major inputs (big contiguous DMA runs).
    xT_d = nc.dram_tensor("xT", [128, SN, CK, 512], BF16, kind="ExternalInput").ap()
    wqk_d = nc.dram_tensor("wqk", [128, 6, CK, 128], BF16, kind="ExternalInput").ap()
    wvT_d = nc.dram_tensor(
        "wvT", [128, CK, H_LOC * D], BF16, kind="ExternalInput"
    ).ap()
    woT_d = nc.dram_tensor("woT", [128, 3, C], BF16, kind="ExternalInput").ap()
    if flip:
        ident_d = nc.dram_tensor("ident", [128, 128], BF16, kind="ExternalInput").ap()
    out_d = nc.dram_tensor("outT_part", [C, S], F32, kind="ExternalOutput").ap()

    EXP = mybir.ActivationFunctionType.Exp
    MULT = mybir.AluOpType.mult

    with tile.TileContext(nc) as tc, ExitStack() as ctx:
        persist = ctx.enter_context(tc.tile_pool(name="persist", bufs=1))
        # PSUM budget (8 banks of 2KB):
        #   spool 2 x [128,1024]f32 = 4 | cpool [*,1024]f32 = 2
        #   bcpool/psT = 1             | aux [128,512]f32 = 1
        spool = ctx.enter_context(tc.tile_pool(name="sc", bufs=2, space="PSUM"))
        cpool = ctx.enter_context(tc.tile_pool(name="cx", bufs=1, space="PSUM"))
        bcpool = ctx.enter_context(tc.tile_pool(name="bc", bufs=1, space="PSUM"))
        aux = ctx.enter_context(tc.tile_pool(name="aux", bufs=1, space="PSUM"))
        ppool = ctx.enter_context(tc.tile_pool(name="probs", bufs=4))
        smallp = ctx.enter_context(tc.tile_pool(name="small", bufs=2))
        outp = ctx.enter_context(tc.tile_pool(name="osb", bufs=8))

        xT_sb = persist.tile([128, SN, CK, 512], BF16, tag="xT")
        wqk_sb = persist.tile([128, 6, CK, 128], BF16, tag="wqk")
        wvT_sb = persist.tile([128, CK, H_LOC * D], BF16, tag="wvT")
        woT_sb = persist.tile([128, 3, C], BF16, tag="woT")
        qkT_sb = persist.tile([128, 6, S], BF16, tag="qkT")
        v_sb = persist.tile([128, SK, H_LOC, D + 1], BF16, tag="vsb")
        ctxT_sb = persist.tile([128, 3, S], BF16, tag="ctxT")
        ones_sb = persist.tile([1, D], BF16, tag="ones")
        if flip:
            ident_sb = persist.tile([128, 128], BF16, tag="ident")

        def emit_body(rep):
            # ---- chunked input DMAs, in consumption order ----
            def dma_wqk(f):
                nc.sync.dma_start(wqk_sb[:, f], wqk_d[:, f])

            def dma_xt(s):
                nc.sync.dma_start(xT_sb[:, s], xT_d[:, s])

            dma_wqk(0)
            dma_xt(0)
            dma_wqk(2)
            nc.sync.dma_start(wvT_sb[:], wvT_d[:])
            dma_xt(1)
            dma_wqk(1)
            dma_wqk(3)
            dma_xt(2)
            dma_xt(3)
            dma_wqk(4)
            dma_wqk(5)
            nc.sync.dma_start(woT_sb[:], woT_d[:])
            if flip:
                nc.sync.dma_start(ident_sb[:], ident_d[:])

            # ones column for the softmax-denominator trick; zero the 64 pad
            # partitions of the last ctxT chunk (head 4 has no pair); ones row
            # for the denominator-broadcast matmul.
            nc.vector.memset(v_sb[:, :, :, D : D + 1], 1.0)
            nc.vector.memset(ctxT_sb[64:128, 2, :], 0.0)
            nc.vector.memset(ones_sb[:], 1.0)

            # ---- unit emitters ----
            def a1(f, s, pool):
                """qkT chunk: [128 feat, 512 seq] = wqk_f^T @ xT_slice."""
                ps = pool.tile([128, 512], F32, tag="sc" if pool is spool else "aux",
                               name=f"a1ps_{rep}_{f}_{s}")
                for c in range(CK):
                    nc.tensor.matmul(
                        ps[:, 0:512],
                        lhsT=wqk_sb[:, f, c, :],
                        rhs=xT_sb[:, s, c, :],
                        start=(c == 0),
                        stop=(c == CK - 1),
                    )
                nc.vector.tensor_copy(
                    out=qkT_sb[:, f, s * 512 : (s + 1) * 512], in_=ps[:, 0:512]
                )

            def a2(m, pool):
                """v chunk: [128 seq, 320 feat] = xT_m^T @ wvT."""
                ps = pool.tile([128, 512], F32, tag="sc" if pool is spool else "aux",
                               name=f"a2ps_{rep}_{m}")
                s, j = m // 4, (m % 4) * 128
                for c in range(CK):
                    nc.tensor.matmul(
                        ps[:, 0 : H_LOC * D],
                        lhsT=xT_sb[:, s, c, j : j + 128],
                        rhs=wvT_sb[:, c, :],
                        start=(c == 0),
                        stop=(c == CK - 1),
                    )
                nc.vector.tensor_copy(
                    out=v_sb[:, m, :, 0:D],
                    in_=ps[:, 0 : H_LOC * D].rearrange("p (h d) -> p h d", h=H_LOC),
                )

            ob_state = {}

            def op(cc, qb, pool, ptag):
                """outT block: [128 cols, 512 seq] = Wo_loc^T-chunk @ ctxT.

                Adjacent qb pairs (0,1) and (2,3) share one [128,1024] SBUF
                staging tile and a single batched DMA, halving the SWDGE
                descriptor-generation load on the Pool queue."""
                ps = pool.tile([128, 512], F32, tag=ptag, name=f"ops_{rep}_{cc}_{qb}")
                for j in range(3):
                    nc.tensor.matmul(
                        ps[:, 0:512],
                        lhsT=woT_sb[:, j, cc * 128 : (cc + 1) * 128],
                        rhs=ctxT_sb[:, j, qb * 512 : (qb + 1) * 512],
                        start=(j == 0),
                        stop=(j == 2),
                    )
                pair = (cc, qb // 2)
                ob = ob_state.pop(pair, None)
                if ob is None:
                    ob = outp.tile(
                        [128, 1024], F32, tag="osb", name=f"ob_{rep}_{cc}_{qb // 2}"
                    )
                    ob_state[pair] = ob
                nc.vector.tensor_copy(
                    out=ob[:, (qb % 2) * 512 : (qb % 2) * 512 + 512], in_=ps[:, 0:512]
                )
                if pair not in ob_state:  # second half done -> flush
                    q0 = (qb // 2) * 1024
                    nc.gpsimd.dma_start(
                        out_d[cc * 128 : (cc + 1) * 128, q0 : q0 + 1024], ob[:]
                    )

            # ---- attention units ----
            def scores_unit(h, sq, sk):
                """scoresT psum [128 k, 1024 q] + exp -> probs bf16."""
                qc, qo = _q_loc(h)
                kc, ko = _k_loc(h)
                sc = spool.tile([128, SQB], F32, tag="sc", name=f"sc_{h}_{sq}_{sk}")
                for n in range(NQ):
                    nc.tensor.matmul(
                        sc[:, n * 512 : (n + 1) * 512],
                        lhsT=qkT_sb[ko : ko + D, kc, sk * 128 : (sk + 1) * 128],
                        rhs=qkT_sb[
                            qo : qo + D,
                            qc,
                            sq * SQB + n * 512 : sq * SQB + (n + 1) * 512,
                        ],
                        start=True,
                        stop=True,
                    )
                pt = ppool.tile([128, SQB], BF16, tag="probs", name=f"pt_{h}_{sq}_{sk}")
                nc.scalar.activation(pt[:, 0:SQB], sc[:, 0:SQB], EXP)
                return pt

            def ctx_unit(h, sq, sk, pt, ctx_ps):
                if flip:
                    # PSUM start zeroes a whole 2KB bank (lazy pending-zero):
                    # only the first qc group of each bank may issue start, the
                    # other three inherit the bank's pending-zero state.
                    for qc in range(SQB // 128):
                        nc.tensor.matmul(
                            ctx_ps[:, qc, 0 : D + 1],
                            lhsT=pt[:, qc * 128 : (qc + 1) * 128],
                            rhs=v_sb[:, sk, h, :],
                            start=(sk == 0 and qc % 4 == 0),
                            stop=(sk == SK - 1 and qc % 4 == 0),
                            skip_group_check=True,
                        )
                else:
                    for n in range(NQ):
                        nc.tensor.matmul(
                            ctx_ps[0 : D + 1, n * 512 : (n + 1) * 512],
                            lhsT=v_sb[:, sk, h, :],
                            rhs=pt[:, n * 512 : (n + 1) * 512],
                            start=(sk == 0),
                            stop=(sk == SK - 1),
                        )

            def nrm(h, sq, ctx_ps):
                """Normalize + store ctxT bf16 for the out-projection."""
                jc, po = h // 2, (h % 2) * 64
                if flip:
                    # per-partition softmax normalize, then PE-transpose back
                    rec = smallp.tile([128, 8], F32, tag="recf", name=f"rec_{h}_{sq}")
                    nc.vector.reciprocal(rec[:], ctx_ps[:, :, D])
                    ctxn = smallp.tile(
                        [128, 8, D], BF16, tag="ctxn", name=f"ctxn_{h}_{sq}"
                    )
                    for qc in range(8):
                        nc.vector.tensor_scalar_mul(
                            ctxn[:, qc, :], ctx_ps[:, qc, 0:D], rec[:, qc : qc + 1]
                        )
                    psT = _flip_state.get("psT")
                    if psT is None:
                        psT = bcpool.tile(
                            [128, SQB], BF16, tag="bc", name=f"psT_{h}_{sq}"
                        )
                        _flip_state["psT"] = psT
                    for qc in range(8):
                        nc.tensor.transpose(
                            psT[po : po + D, qc * 128 : (qc + 1) * 128],
                            ctxn[:, qc, :],
                            ident_sb[:],
                        )
                    if h % 2 == 1 or h == 4:
                        rows = 128 if h % 2 == 1 else 64
                        nc.vector.tensor_copy(
                            out=ctxT_sb[0:rows, jc, sq * SQB : (sq + 1) * SQB],
                            in_=psT[0:rows, :],
                        )
                        _flip_state["psT"] = None
                else:
                    rec = smallp.tile([1, SQB], BF16, tag="rec", name=f"rec_{h}_{sq}")
                    with nc.allow_low_precision(reason="softmax recip bf16"):
                        nc.vector.reciprocal(rec[:], ctx_ps[D : D + 1, 0:SQB])
                    for half in range(2):
                        c0 = half * 512
                        bc = bcpool.tile(
                            [D, 512], F32, tag="bc", name=f"bc_{h}_{sq}_{half}"
                        )
                        nc.tensor.matmul(
                            bc[:, 0:512],
                            lhsT=ones_sb[0:1, :],
                            rhs=rec[0:1, c0 : c0 + 512],
                            start=True,
                            stop=True,
                        )
                        # DVE cannot read two PSUM operands; stage bc in SBUF
                        bcs = smallp.tile(
                            [D, 512], BF16, tag="bcs", name=f"bcs_{h}_{sq}_{half}"
                        )
                        nc.vector.tensor_copy(out=bcs[:], in_=bc[:, 0:512])
                        nc.vector.tensor_tensor(
                            out=ctxT_sb[
                                po : po + D, jc, sq * SQB + c0 : sq * SQB + c0 + 512
                            ],
                            in0=ctx_ps[0:D, c0 : c0 + 512],
                            in1=bcs[:],
                            op=MULT,
                        )

            _flip_state = {"psT": None}

            # ---- lead-in: enough projection work to start attention ----
            a1(0, 0, spool)
            a1(0, 1, spool)
            a1(2, 0, spool)
            a2(0, spool)
            a2(1, spool)

            # ---- filler schedule: (sq, h) -> 16 per-iteration buckets.
            # Ordering constraints: scores(h, sq, sk) needs k-chunk slice
            # floor(sk/4) of the head's k block BEFORE iteration sk, and
            # ctx(h, sq, m) (emitted at iteration m+1, before fillers) needs
            # a2(m) emitted at iteration <= m.
            def A1(f, s):
                return lambda: a1(f, s, aux)

            def A2(m):
                return lambda: a2(m, aux)

            def OP(c, q):
                return lambda: op(c, q, aux, "aux")

            def spread(fillers, step=2, start=0):
                buckets = [[] for _ in range(SK)]
                for i, f in enumerate(fillers):
                    buckets[min(start + i * step, SK - 1)].append(f)
                return buckets

            fill = {
                (0, 0): [
                    [A1(2, 1), A2(2)], [A2(3)], [A2(4)],
                    [A1(2, 2), A2(5)], [A2(6)], [A2(7)],
                    [A1(2, 3), A2(8)], [A2(9)], [A2(10)], [A2(11)], [A2(12)],
                    [A2(13)], [A2(14)], [A2(15)], [], [],
                ],
                (0, 1): spread(
                    [A1(1, 0), A1(3, 0), A1(1, 1), A1(3, 1), A1(3, 2), A1(3, 3)]
                ),
                (0, 2): spread(
                    [A1(4, 0), A1(5, 0), A1(4, 1), A1(5, 1), A1(5, 2), A1(5, 3)]
                ),
                (0, 3): spread([A1(0, 2), A1(0, 3), A1(1, 2), A1(1, 3)]),
                (0, 4): spread([A1(4, 2), A1(4, 3)]),
                # start=2: sq0's ctxT is only complete after the deferred
                # nrm of (0,4), which is emitted at iteration sk==1 here
                (1, 0): spread(
                    [OP(c, q) for c in range(4) for q in range(2)], start=2
                ),
                (1, 1): spread([OP(c, q) for c in range(4, 8) for q in range(2)]),
                (1, 2): spread([OP(c, q) for c in range(8, 10) for q in range(2)]),
                (1, 3): [[] for _ in range(SK)],
                (1, 4): [[] for _ in range(SK)],
            }

            # ---- main attention stream (sq-outer), software-pipelined ----
            prev_nrm = None  # deferred normalize of the previous head
            ctx_shape = [128, 8, 128] if flip else [128, SQB]
            for sq in range(NSQ):
                for h in range(H_LOC):
                    buckets = fill[(sq, h)]
                    # ctx tile is allocated lazily AFTER the previous head's
                    # deferred normalize is emitted: cpool has bufs=1, so the
                    # slot must have all its readers emitted before reuse.
                    ctx_ps = None
                    pts = {}
                    for sk in range(SK):
                        pts[sk] = scores_unit(h, sq, sk)
                        if sk == 1 and prev_nrm is not None:
                            # normalize the previous head once this head's
                            # first scores are in flight (frees its ctx tile
                            # well before our first ctx matmul needs it)
                            prev_nrm()
                            prev_nrm = None
                        if sk > 0:
                            if ctx_ps is None:
                                ctx_ps = cpool.tile(
                                    ctx_shape, F32, tag="ctx", name=f"ctx_{h}_{sq}"
                                )
                            ctx_unit(h, sq, sk - 1, pts.pop(sk - 1), ctx_ps)
                        for f in buckets[sk]:
                            f()
                    ctx_unit(h, sq, SK - 1, pts.pop(SK - 1), ctx_ps)
                    prev_nrm = (lambda hh, ss, cp: lambda: nrm(hh, ss, cp))(
                        h, sq, ctx_ps
                    )
            prev_nrm()

            # ---- tail: remaining out-proj blocks, rotating over freed pools ----
            pools = [(spool, "sc"), (cpool, "ctx"), (aux, "aux"), (spool, "sc")]
            i = 0
            for cc in range(CCH):
                for qb in (2, 3):
                    p, t = pools[i % len(pools)]
                    op(cc, qb, p, t)
                    i += 1

        for rep in range(repeat):
            emit_body(rep)

    nc.compile()
    return nc


def make_core_inputs(x, Wq_eff, Wk_eff, Wv_eff, Wo_eff):
    """Per-core input dicts. x [B,S,C] f32; W_eff [C,C] f32 (scale folded).

    All tensors are pre-shuffled into the kernel's partition-major DMA
    layouts so every chunk DMA moves large contiguous runs.
    """
    B, S, C = x.shape
    CK = C // 128
    in_maps = []
    xT_p = []
    for b in range(B):
        xT = np.ascontiguousarray(x[b].T).astype(NPBF16)  # [C, S]
        xT_p.append(
            np.ascontiguousarray(
                xT.reshape(CK, 128, 4, 512).transpose(1, 2, 0, 3)
            )
        )  # [128, 4, CK, 512]
    ident = np.eye(128, dtype=np.float32).astype(NPBF16)
    for core in range(N_CORES):
        b, g = core // 4, core % 4
        r0 = g * H_LOC * D  # first feature row of this core's heads
        qf = Wq_eff[r0 : r0 + H_LOC * D]  # (320, C)
        kf = Wk_eff[r0 : r0 + H_LOC * D]
        vf = Wv_eff[r0 : r0 + H_LOC * D]
        zero = np.zeros((D, C), np.float32)
        # chunks: (q0,q1)(q2,q3)(k0,k1)(k2,k3)(q4,0)(k4,0)
        wqk = np.concatenate(
            [qf[: 4 * D], kf[: 4 * D], qf[4 * D :], zero, kf[4 * D :], zero], axis=0
        ).T  # (C, 768)
        wqk_p = (
            wqk.astype(NPBF16).reshape(CK, 128, 6, 128).transpose(1, 2, 0, 3)
        )  # [128, 6, CK, 128]
        wvT_p = (
            vf.T.astype(NPBF16).reshape(CK, 128, H_LOC * D).transpose(1, 0, 2)
        )  # [128, CK, 320]
        woT = np.concatenate(
            [Wo_eff[:, r0 : r0 + H_LOC * D].T, np.zeros((D, C), np.float32)], axis=0
        )  # (384, C)
        woT_p = woT.astype(NPBF16).reshape(3, 128, C).transpose(1, 0, 2)  # [128,3,C]
        in_maps.append(
            {
                "xT": xT_p[b],
                "wqk": np.ascontiguousarray(wqk_p),
                "wvT": np.ascontiguousarray(wvT_p),
                "woT": np.ascontiguousarray(woT_p),
                "ident": ident,
            }
        )
    return in_maps


def fold_weights(Wq, Wk, Wv, Wo, Aq, Bq, Ak, Bk, Av, Bv, Ao, Bo):
    scale = 1.0 / np.sqrt(np.float32(D))
    Wq_eff = (Wq + Bq @ Aq) * scale
    Wk_eff = Wk + Bk @ Ak
    Wv_eff = Wv + Bv @ Av
    Wo_eff = Wo + Bo @ Ao
    return Wq_eff, Wk_eff, Wv_eff, Wo_eff


_NC_CACHE = {}


def _get_program(S, C):
    key = (S, C)
    if key not in _NC_CACHE:
        _NC_CACHE[key] = build_program(S, C)
    return _NC_CACHE[key]


def kernel(**inputs):
    inputs = {k: np.asarray(v, np.float32) for k, v in inputs.items()}
    x = inputs["x"]
    B, S, C = x.shape
    Wq_eff, Wk_eff, Wv_eff, Wo_eff = fold_weights(
        inputs["Wq"], inputs["Wk"], inputs["Wv"], inputs["Wo"],
        inputs["Aq"], inputs["Bq"], inputs["Ak"], inputs["Bk"],
        inputs["Av"], inputs["Bv"], inputs["Ao"], inputs["Bo"],
    )
    in_maps = make_core_inputs(x, Wq_eff, Wk_eff, Wv_eff, Wo_eff)
    nc = _get_program(S, C)
    if not FLIP:
        for m in in_maps:
            m.pop("ident", None)  # non-flip program has no ident input
    res = run_bass_kernel_spmd(nc, in_maps, list(range(N_CORES)))
    parts = [res.results[c]["outT_part"].astype(np.float32) for c in range(N_CORES)]
    bo = inputs["bo"]
    out = np.stack(
        [
            (parts[0] + parts[1] + parts[2] + parts[3]).T + bo,
            (parts[4] + parts[5] + parts[6] + parts[7]).T + bo,
        ]
    ).astype(np.float32)
    return out


# revision 49
# speedup vs baseline: 1.5548x; 1.4919x over previous
"""Trainium2 Bass kernel for LoRA self-attention (nn_LoRAAttnProcessor).

Problem shapes (hardcoded): x [2, 2048, 1280], 20 heads x 64 dim, LoRA rank 4.

Strategy
--------
* Host side: fold every LoRA pair into its base weight (W_eff = W + B @ A) --
  mathematically identical (associativity), and fold the 1/sqrt(D) score
  scale into Wq_eff.  The kernel then computes plain multi-head attention.
* Sharding: 8 cores x (batch b = core//4, 5 heads = core%4).  Wq/Wk/Wv are
  column-sharded by head, Wo row-sharded by head; each core emits a partial
  TRANSPOSED output [1280, 2048] that the host sums per batch element and
  transposes (+ bias bo).
* Per core (S=2048, C=1280, 5 local heads, D=64), all matmuls in bf16 with
  fp32 PSUM accumulation.  Schedule: chunked input DMAs (host pre-shuffles
  weights/activations into partition-major blocks so every DMA moves large
  contiguous runs), software pipelining of the projection work (A1 qk-proj,
  A2 v-proj, transposed out-proj) into the attention phase, dedicated PSUM
  pools (scores x2 | ctx | bcast | aux = 8 banks exactly), matmul-based
  softmax-denominator broadcast, and output DMAs issued from the otherwise
  idle GPSIMD queue so they never serialize against next-iteration input
  DMAs on the SP queue.
"""

import os
import sys

if "/opt/trn_rl_repo" not in sys.path:
    sys.path.insert(0, "/opt/trn_rl_repo")

from contextlib import ExitStack

import ml_dtypes
import numpy as np

import concourse.bass as bass
import concourse.tile as tile
from concourse import bacc, mybir
from concourse.bass_utils import run_bass_kernel_spmd

BF16 = mybir.dt.bfloat16
F32 = mybir.dt.float32
NPBF16 = ml_dtypes.bfloat16

D = 64
H_LOC = 5  # heads per core
N_CORES = 8


def _q_loc(h):
    """(chunk, partition offset) of qT for local head h in qkT_sb."""
    return (h // 2, (h % 2) * 64) if h < 4 else (4, 0)


def _k_loc(h):
    return (2 + h // 2, (h % 2) * 64) if h < 4 else (5, 0)


# flip=True (q-major ctx accumulation) measures ~24% faster than the
# k-major layout under the widened repeat-differential (312us vs 408us
# per body): the 8 thin N=65 ctx matmuls halve PE stream cycles and the
# per-partition softmax normalize removes the broadcast machinery.
FLIP = os.environ.get("KFLIP", "1") == "1"


def build_program(S=2048, C=1280, paired=False, interleave=False, repeat=1,
                  flip=None):
    """Build the SPMD single-core program. S == 2048, C == 1280.

    flip=False: ctx accumulated k-major as ctxT [65, q] (2 fat matmuls/step),
    softmax denominator broadcast via ones-matmul.
    flip=True: ctx accumulated q-major as [q, 65] (8 thin matmuls/step, half
    the PE stream cycles), per-partition normalize, PE-transpose back to ctxT.
    """
    if flip is None:
        flip = FLIP
    assert S == 2048 and C % 128 == 0
    CK = C // 128         # contraction chunks over channels (10)
    SN = S // 512         # 512-col slices of sequence (4)
    SQB = 1024            # query block width
    NSQ = S // SQB        # 2
    SK = S // 128         # key chunks (16)
    NQ = SQB // 512       # 2
    CCH = C // 128        # out-proj column chunks (10)

    nc = bacc.Bacc("TRN2", target_bir_lowering=False, debug=False)

    # Host pre-shuffled, partition-major inputs (big contiguous DMA runs).
    xT_d = nc.dram_tensor("xT", [128, SN, CK, 512], BF16, kind="ExternalInput").ap()
    wqk_d = nc.dram_tensor("wqk", [128, 5, CK, 128], BF16, kind="ExternalInput").ap()
    wvT_d = nc.dram_tensor(
        "wvT", [128, CK, H_LOC * D], BF16, kind="ExternalInput"
    ).ap()
    woT_d = nc.dram_tensor("woT", [128, 3, C], BF16, kind="ExternalInput").ap()
    if flip:
        ident_d = nc.dram_tensor("ident", [128, 128], BF16, kind="ExternalInput").ap()
    out_d = nc.dram_tensor("outT_part", [C, S], F32, kind="ExternalOutput").ap()

    EXP = mybir.ActivationFunctionType.Exp
    MULT = mybir.AluOpType.mult

    with tile.TileContext(nc) as tc, ExitStack() as ctx:
        persist = ctx.enter_context(tc.tile_pool(name="persist", bufs=1))
        # PSUM budget (8 banks of 2KB):
        #   spool 2 x [128,1024]f32 = 4 | cpool [*,1024]f32 = 2
        #   bcpool/psT = 1             | aux [128,512]f32 = 1
        spool = ctx.enter_context(tc.tile_pool(name="sc", bufs=2, space="PSUM"))
        cpool = ctx.enter_context(tc.tile_pool(name="cx", bufs=1, space="PSUM"))
        bcpool = ctx.enter_context(tc.tile_pool(name="bc", bufs=1, space="PSUM"))
        aux = ctx.enter_context(tc.tile_pool(name="aux", bufs=1, space="PSUM"))
        ppool = ctx.enter_context(tc.tile_pool(name="probs", bufs=4))
        smallp = ctx.enter_context(tc.tile_pool(name="small", bufs=2))
        outp = ctx.enter_context(tc.tile_pool(name="osb", bufs=8))

        # xT is double-buffered by repetition parity so the next iteration's
        # input DMAs can start while this iteration still reads the current
        # buffer (steady-state cross-iteration pipelining)
        xT_bufs = [
            persist.tile([128, SN, CK, 512], BF16, tag="xT0", name="xT_sb0"),
            persist.tile([128, SN, CK, 512], BF16, tag="xT1", name="xT_sb1"),
        ]
        wqk_sb = persist.tile([128, 5, CK, 128], BF16, tag="wqk")
        wvT_sb = persist.tile([128, CK, H_LOC * D], BF16, tag="wvT")
        woT_sb = persist.tile([128, 3, C], BF16, tag="woT")
        qkT_sb = persist.tile([128, 6, S], BF16, tag="qkT")
        v_sb = persist.tile([128, SK, H_LOC, D + 1], BF16, tag="vsb")
        ctxT_sb = persist.tile([128, 3, S], BF16, tag="ctxT")
        ones_sb = persist.tile([1, D], BF16, tag="ones")
        if flip:
            ident_sb = persist.tile([128, 128], BF16, tag="ident")

        # ---- build-level unit emitters (rep-parametric) ----
        def prologue(rep):
            """Chunked input DMAs for iteration `rep`, in consumption order.
            xT goes to the parity buffer; weight buffers are single and
            their WAR dependencies have cleared by the time these fire."""
            xT_sb = xT_bufs[rep % 2]

            def dma_wqk(f):
                nc.sync.dma_start(wqk_sb[:, f], wqk_d[:, f])

            dma_wqk(0)
            nc.sync.dma_start(xT_sb[:, 0], xT_d[:, 0])
            dma_wqk(2)
            nc.sync.dma_start(wvT_sb[:], wvT_d[:])
            nc.sync.dma_start(xT_sb[:, 1], xT_d[:, 1])
            dma_wqk(1)
            dma_wqk(3)
            nc.sync.dma_start(xT_sb[:, 2], xT_d[:, 2])
            nc.sync.dma_start(xT_sb[:, 3], xT_d[:, 3])
            dma_wqk(4)
            nc.sync.dma_start(woT_sb[:], woT_d[:])

        def a1(rep, f, s, pool):
            """qkT chunk: [128 feat, 512 seq] = wqk_f^T @ xT_slice.

            Chunk 4 is packed [q4 | k4]; its top half is split out to
            qkT chunk 5 partitions 0:64 with a partition-shifted copy, so
            no zero-padded projection chunks are ever streamed."""
            xT_sb = xT_bufs[rep % 2]
            ps = pool.tile([128, 512], F32, tag="sc" if pool is spool else "aux",
                           name=f"a1ps_{rep}_{f}_{s}")
            for c in range(CK):
                nc.tensor.matmul(
                    ps[:, 0:512],
                    lhsT=wqk_sb[:, f, c, :],
                    rhs=xT_sb[:, s, c, :],
                    start=(c == 0),
                    stop=(c == CK - 1),
                )
            if f == 4:
                nc.vector.tensor_copy(
                    out=qkT_sb[0:64, 4, s * 512 : (s + 1) * 512],
                    in_=ps[0:64, 0:512],
                )
                nc.vector.tensor_copy(
                    out=qkT_sb[0:64, 5, s * 512 : (s + 1) * 512],
                    in_=ps[64:128, 0:512],
                )
            else:
                nc.vector.tensor_copy(
                    out=qkT_sb[:, f, s * 512 : (s + 1) * 512], in_=ps[:, 0:512]
                )

        def a2(rep, m, pool):
            """v chunk: [128 seq, 320 feat] = xT_m^T @ wvT."""
            xT_sb = xT_bufs[rep % 2]
            ps = pool.tile([128, 512], F32, tag="sc" if pool is spool else "aux",
                           name=f"a2ps_{rep}_{m}")
            s, j = m // 4, (m % 4) * 128
            for c in range(CK):
                nc.tensor.matmul(
                    ps[:, 0 : H_LOC * D],
                    lhsT=xT_sb[:, s, c, j : j + 128],
                    rhs=wvT_sb[:, c, :],
                    start=(c == 0),
                    stop=(c == CK - 1),
                )
            nc.vector.tensor_copy(
                out=v_sb[:, m, :, 0:D],
                in_=ps[:, 0 : H_LOC * D].rearrange("p (h d) -> p h d", h=H_LOC),
            )

        def lead_units(rep, pool):
            """The projection units that must precede iteration `rep`'s
            attention stream: q01/k01 first chunks + first two v chunks."""
            return [
                lambda: a1(rep, 0, 0, pool),
                lambda: a1(rep, 0, 1, pool),
                lambda: a1(rep, 2, 0, pool),
                lambda: a2(rep, 0, pool),
                lambda: a2(rep, 1, pool),
            ]

        no_inject = os.environ.get("KNOINJ", "0") == "1"

        def emit_body(rep):
            if rep == 0:
                if flip:
                    nc.sync.dma_start(ident_sb[:], ident_d[:])
                # ones column for the softmax-denominator trick; zero the 64
                # pad partitions of the last ctxT chunk (head 4 has no pair);
                # ones row for the denominator-broadcast matmul. Set once --
                # no iteration overwrites them.
                nc.vector.memset(v_sb[:, :, :, D : D + 1], 1.0)
                nc.vector.memset(ctxT_sb[64:128, 2, :], 0.0)
                nc.vector.memset(ones_sb[:], 1.0)
            if rep == 0 or no_inject:
                prologue(rep)
                for u in lead_units(rep, spool):
                    u()
            # (otherwise the prologue and lead were injected into the
            # previous iteration's late-sq1 filler buckets)

            ob_state = {}
            COPY = mybir.ActivationFunctionType.Copy

            def op(cc, qb, pool, ptag, act_copy=False):
                """outT block: [128 cols, 512 seq] = Wo_loc^T-chunk @ ctxT.

                Adjacent qb pairs (0,1) and (2,3) share one [128,1024] SBUF
                staging tile and a single batched DMA, halving the SWDGE
                descriptor-generation load on the Pool queue."""
                ps = pool.tile([128, 512], F32, tag=ptag, name=f"ops_{rep}_{cc}_{qb}")
                for j in range(3):
                    nc.tensor.matmul(
                        ps[:, 0:512],
                        lhsT=woT_sb[:, j, cc * 128 : (cc + 1) * 128],
                        rhs=ctxT_sb[:, j, qb * 512 : (qb + 1) * 512],
                        start=(j == 0),
                        stop=(j == 2),
                    )
                pair = (cc, qb // 2)
                ob = ob_state.pop(pair, None)
                if ob is None:
                    ob = outp.tile(
                        [128, 1024], F32, tag="osb", name=f"ob_{rep}_{cc}_{qb // 2}"
                    )
                    ob_state[pair] = ob
                dst = ob[:, (qb % 2) * 512 : (qb % 2) * 512 + 512]
                if act_copy:
                    # tail only: ACT is idle there, share the copy load
                    nc.scalar.activation(dst, ps[:, 0:512], COPY)
                else:
                    nc.vector.tensor_copy(out=dst, in_=ps[:, 0:512])
                if pair not in ob_state:  # second half done -> flush
                    q0 = (qb // 2) * 1024
                    nc.gpsimd.dma_start(
                        out_d[cc * 128 : (cc + 1) * 128, q0 : q0 + 1024], ob[:]
                    )

            # ---- attention units ----
            def scores_unit(h, sq, sk):
                """scoresT psum [128 k, 1024 q] + exp -> probs bf16."""
                qc, qo = _q_loc(h)
                kc, ko = _k_loc(h)
                sc = spool.tile([128, SQB], F32, tag="sc", name=f"sc_{h}_{sq}_{sk}")
                for n in range(NQ):
                    nc.tensor.matmul(
                        sc[:, n * 512 : (n + 1) * 512],
                        lhsT=qkT_sb[ko : ko + D, kc, sk * 128 : (sk + 1) * 128],
                        rhs=qkT_sb[
                            qo : qo + D,
                            qc,
                            sq * SQB + n * 512 : sq * SQB + (n + 1) * 512,
                        ],
                        start=True,
                        stop=True,
                    )
                pt = ppool.tile([128, SQB], BF16, tag="probs", name=f"pt_{h}_{sq}_{sk}")
                nc.scalar.activation(pt[:, 0:SQB], sc[:, 0:SQB], EXP)
                return pt

            def ctx_unit(h, sq, sk, pt, ctx_ps):
                if flip:
                    # PSUM start zeroes a whole 2KB bank (lazy pending-zero):
                    # only the first qc group of each bank may issue start, the
                    # other three inherit the bank's pending-zero state.
                    for qc in range(SQB // 128):
                        nc.tensor.matmul(
                            ctx_ps[:, qc, 0 : D + 1],
                            lhsT=pt[:, qc * 128 : (qc + 1) * 128],
                            rhs=v_sb[:, sk, h, :],
                            start=(sk == 0 and qc % 4 == 0),
                            stop=(sk == SK - 1 and qc % 4 == 0),
                            skip_group_check=True,
                        )
                else:
                    for n in range(NQ):
                        nc.tensor.matmul(
                            ctx_ps[0 : D + 1, n * 512 : (n + 1) * 512],
                            lhsT=v_sb[:, sk, h, :],
                            rhs=pt[:, n * 512 : (n + 1) * 512],
                            start=(sk == 0),
                            stop=(sk == SK - 1),
                        )

            def nrm_a(h, sq, ctx_ps):
                """Flip phase A: per-partition softmax normalize -> ctxn bf16.
                Frees the ctx PSUM slot; the PE-transposes happen in nrm_b,
                deferred further so they never wait on this DVE chain."""
                rec = smallp.tile([128, 8], F32, tag="recf", name=f"rec_{h}_{sq}")
                nc.vector.reciprocal(rec[:], ctx_ps[:, :, D])
                ctxn = smallp.tile(
                    [128, 8, D], BF16, tag=f"ctxn{h % 2}", name=f"ctxn_{h}_{sq}"
                )
                for qc in range(8):
                    nc.vector.tensor_scalar_mul(
                        ctxn[:, qc, :], ctx_ps[:, qc, 0:D], rec[:, qc : qc + 1]
                    )
                return ctxn

            def nrm_b(h, sq, ctxn):
                """Flip phase B: PE-transpose ctxn back to ctxT layout."""
                jc, po = h // 2, (h % 2) * 64
                psT = _flip_state.get("psT")
                if psT is None:
                    psT = bcpool.tile(
                        [128, SQB], BF16, tag="bc", name=f"psT_{h}_{sq}"
                    )
                    _flip_state["psT"] = psT
                for qc in range(8):
                    nc.tensor.transpose(
                        psT[po : po + D, qc * 128 : (qc + 1) * 128],
                        ctxn[:, qc, :],
                        ident_sb[:],
                    )
                if h % 2 == 1 or h == 4:
                    rows = 128 if h % 2 == 1 else 64
                    nc.vector.tensor_copy(
                        out=ctxT_sb[0:rows, jc, sq * SQB : (sq + 1) * SQB],
                        in_=psT[0:rows, :],
                    )
                    _flip_state["psT"] = None

            def nrm(h, sq, ctx_ps):
                """Normalize + store ctxT bf16 for the out-projection."""
                jc, po = h // 2, (h % 2) * 64
                if flip:
                    nrm_b(h, sq, nrm_a(h, sq, ctx_ps))
                else:
                    rec = smallp.tile([1, SQB], BF16, tag="rec", name=f"rec_{h}_{sq}")
                    with nc.allow_low_precision(reason="softmax recip bf16"):
                        nc.vector.reciprocal(rec[:], ctx_ps[D : D + 1, 0:SQB])
                    for half in range(2):
                        c0 = half * 512
                        bc = bcpool.tile(
                            [D, 512], F32, tag="bc", name=f"bc_{h}_{sq}_{half}"
                        )
                        nc.tensor.matmul(
                            bc[:, 0:512],
                            lhsT=ones_sb[0:1, :],
                            rhs=rec[0:1, c0 : c0 + 512],
                            start=True,
                            stop=True,
                        )
                        # DVE cannot read two PSUM operands; stage bc in SBUF
                        bcs = smallp.tile(
                            [D, 512], BF16, tag="bcs", name=f"bcs_{h}_{sq}_{half}"
                        )
                        nc.vector.tensor_copy(out=bcs[:], in_=bc[:, 0:512])
                        nc.vector.tensor_tensor(
                            out=ctxT_sb[
                                po : po + D, jc, sq * SQB + c0 : sq * SQB + c0 + 512
                            ],
                            in0=ctx_ps[0:D, c0 : c0 + 512],
                            in1=bcs[:],
                            op=MULT,
                        )

            _flip_state = {"psT": None}

            # ---- filler schedule: (sq, h) -> 16 per-iteration buckets.
            # Ordering constraints: scores(h, sq, sk) needs k-chunk slice
            # floor(sk/4) of the head's k block BEFORE iteration sk, and
            # ctx(h, sq, m) (emitted at iteration m+2, before fillers) needs
            # a2(m) emitted at iteration <= m+1.
            def A1(f, s):
                return lambda: a1(rep, f, s, aux)

            def A2(m):
                return lambda: a2(rep, m, aux)

            def OP(c, q):
                return lambda: op(c, q, aux, "aux")

            def spread(fillers, step=2, start=0):
                buckets = [[] for _ in range(SK)]
                for i, f in enumerate(fillers):
                    buckets[min(start + i * step, SK - 1)].append(f)
                return buckets

            fill = {
                (0, 0): [
                    [A1(2, 1), A2(2)], [A2(3)], [A2(4)],
                    [A1(2, 2), A2(5)], [A2(6)], [A2(7)],
                    [A1(2, 3), A2(8)], [A2(9)], [A2(10)], [A2(11)], [A2(12)],
                    [A2(13)], [A2(14)], [A2(15)], [], [],
                ],
                (0, 1): spread([A1(1, 0), A1(3, 0), A1(1, 1)], step=3),
                # f3 slices must land before h2's own sk reaches them
                # (slice s consumed from iteration 4*s; fillers run after
                # that iteration's scores), f4 before h4 uses them
                (0, 2): [
                    [A1(3, 1)], [], [], [A1(4, 0)], [], [], [A1(3, 2)], [],
                    [], [A1(4, 1)], [], [A1(3, 3)], [], [], [], [],
                ],
                # chunk-4 slices carry k4's sk range too, so they must all
                # land before (0,4) reaches sk 8 / sk 12; pure-q sq1 slices
                # (f0, f1) are deferred as late as their consumers allow
                (0, 3): spread([A1(4, 2), A1(4, 3), A1(0, 2), A1(0, 3)], step=4),
                (0, 4): [[] for _ in range(SK)],
                # start=5: sq0's ctxT is only complete after the deferred
                # nrm_b of (0,4), which is emitted at iteration sk==4 here
                (1, 0): spread(
                    [A1(1, 2), A1(1, 3)]
                    + [OP(c, q) for c in range(0, 2) for q in range(2)],
                    step=2, start=5,
                ),
                (1, 1): spread(
                    [OP(c, q) for c in range(2, 4) for q in range(2)], step=3
                ),
                (1, 2): spread(
                    [OP(c, q) for c in range(4, 6) for q in range(2)], step=3
                ),
                (1, 3): spread(
                    [OP(c, q) for c in range(6, 8) for q in range(2)], step=3
                ),
                (1, 4): spread(
                    [OP(c, q) for c in range(8, 10) for q in range(2)], step=3
                ),
            }

            # ---- cross-iteration pipelining: inject the NEXT iteration's
            # input DMAs and lead projection units into this iteration's
            # ACT-paced late-sq1 region, where the PE otherwise idles ----
            inject = {}
            if rep + 1 < repeat and not no_inject:
                nl = lead_units(rep + 1, aux)
                inject = {
                    (1, 3): {0: [lambda: prologue(rep + 1)], 8: [nl[0]],
                             11: [nl[1]], 14: [nl[2]]},
                    (1, 4): {3: [nl[3]], 6: [nl[4]]},
                }

            # ---- main attention stream (sq-outer), software-pipelined ----
            prev_nrm = None   # deferred normalize (phase A) of previous head
            prev_nrmb = None  # deferred transpose-back (phase B)
            ctx_shape = [128, 8, 128] if flip else [128, SQB]
            for sq in range(NSQ):
                for h in range(H_LOC):
                    buckets = fill[(sq, h)]
                    injections = inject.get((sq, h), {})
                    # ctx tile is allocated lazily AFTER the previous head's
                    # deferred normalize is emitted: cpool has bufs=1, so the
                    # slot must have all its readers emitted before reuse.
                    ctx_ps = None
                    pts = {}
                    # depth-2 software pipeline: ctx(sk-2) after scores(sk),
                    # so the PE never waits on exp(sk-1)'s semaphore
                    for sk in range(SK):
                        pts[sk] = scores_unit(h, sq, sk)
                        if sk == 1 and prev_nrm is not None:
                            # normalize the previous head once this head's
                            # first scores are in flight (frees its ctx tile
                            # well before our first ctx matmul needs it)
                            prev_nrm()
                            prev_nrm = None
                        if sk == 4 and prev_nrmb is not None:
                            # transpose-back runs once the normalize's DVE
                            # chain has surely drained
                            prev_nrmb()
                            prev_nrmb = None
                        if sk > 1:
                            if ctx_ps is None:
                                ctx_ps = cpool.tile(
                                    ctx_shape, F32, tag="ctx", name=f"ctx_{h}_{sq}"
                                )
                            ctx_unit(h, sq, sk - 2, pts.pop(sk - 2), ctx_ps)
                        for f in buckets[sk]:
                            f()
                        for f in injections.get(sk, []):
                            f()
                    ctx_unit(h, sq, SK - 2, pts.pop(SK - 2), ctx_ps)
                    ctx_unit(h, sq, SK - 1, pts.pop(SK - 1), ctx_ps)
                    if flip:
                        def mk(hh, ss, cp):
                            state = {}

                            def pa():
                                state["ctxn"] = nrm_a(hh, ss, cp)

                            def pb():
                                nrm_b(hh, ss, state["ctxn"])

                            return pa, pb

                        prev_nrm, prev_nrmb = mk(h, sq, ctx_ps)
                    else:
                        prev_nrm = (lambda hh, ss, cp: lambda: nrm(hh, ss, cp))(
                            h, sq, ctx_ps
                        )
            prev_nrm()
            if prev_nrmb is not None:
                prev_nrmb()

            # ---- tail: remaining out-proj blocks, rotating over freed pools ----
            pools = [(spool, "sc"), (cpool, "ctx"), (aux, "aux"), (spool, "sc")]
            i = 0
            for cc in range(CCH):
                for qb in (2, 3):
                    p, t = pools[i % len(pools)]
                    op(cc, qb, p, t, act_copy=(i % 2 == 1))
                    i += 1

        for rep in range(repeat):
            emit_body(rep)

    nc.compile()
    return nc


def make_core_inputs(x, Wq_eff, Wk_eff, Wv_eff, Wo_eff):
    """Per-core input dicts. x [B,S,C] f32; W_eff [C,C] f32 (scale folded).

    All tensors are pre-shuffled into the kernel's partition-major DMA
    layouts so every chunk DMA moves large contiguous runs.
    """
    B, S, C = x.shape
    CK = C // 128
    in_maps = []
    xT_p = []
    for b in range(B):
        xT = np.ascontiguousarray(x[b].T).astype(NPBF16)  # [C, S]
        xT_p.append(
            np.ascontiguousarray(
                xT.reshape(CK, 128, 4, 512).transpose(1, 2, 0, 3)
            )
        )  # [128, 4, CK, 512]
    ident = np.eye(128, dtype=np.float32).astype(NPBF16)
    for core in range(N_CORES):
        b, g = core // 4, core % 4
        r0 = g * H_LOC * D  # first feature row of this core's heads
        qf = Wq_eff[r0 : r0 + H_LOC * D]  # (320, C)
        kf = Wk_eff[r0 : r0 + H_LOC * D]
        vf = Wv_eff[r0 : r0 + H_LOC * D]
        # chunks: (q0,q1)(q2,q3)(k0,k1)(k2,k3)(q4,k4) -- no zero padding;
        # the kernel splits chunk 4's halves with a partition-shifted copy
        wqk = np.concatenate(
            [qf[: 4 * D], kf[: 4 * D], qf[4 * D :], kf[4 * D :]], axis=0
        ).T  # (C, 640)
        wqk_p = (
            wqk.astype(NPBF16).reshape(CK, 128, 5, 128).transpose(1, 2, 0, 3)
        )  # [128, 5, CK, 128]
        wvT_p = (
            vf.T.astype(NPBF16).reshape(CK, 128, H_LOC * D).transpose(1, 0, 2)
        )  # [128, CK, 320]
        woT = np.concatenate(
            [Wo_eff[:, r0 : r0 + H_LOC * D].T, np.zeros((D, C), np.float32)], axis=0
        )  # (384, C)
        woT_p = woT.astype(NPBF16).reshape(3, 128, C).transpose(1, 0, 2)  # [128,3,C]
        in_maps.append(
            {
                "xT": xT_p[b],
                "wqk": np.ascontiguousarray(wqk_p),
                "wvT": np.ascontiguousarray(wvT_p),
                "woT": np.ascontiguousarray(woT_p),
                "ident": ident,
            }
        )
    return in_maps


def fold_weights(Wq, Wk, Wv, Wo, Aq, Bq, Ak, Bk, Av, Bv, Ao, Bo):
    scale = 1.0 / np.sqrt(np.float32(D))
    Wq_eff = (Wq + Bq @ Aq) * scale
    Wk_eff = Wk + Bk @ Ak
    Wv_eff = Wv + Bv @ Av
    Wo_eff = Wo + Bo @ Ao
    return Wq_eff, Wk_eff, Wv_eff, Wo_eff


_NC_CACHE = {}


def _get_program(S, C):
    key = (S, C)
    if key not in _NC_CACHE:
        _NC_CACHE[key] = build_program(S, C)
    return _NC_CACHE[key]


def kernel(**inputs):
    inputs = {k: np.asarray(v, np.float32) for k, v in inputs.items()}
    x = inputs["x"]
    B, S, C = x.shape
    Wq_eff, Wk_eff, Wv_eff, Wo_eff = fold_weights(
        inputs["Wq"], inputs["Wk"], inputs["Wv"], inputs["Wo"],
        inputs["Aq"], inputs["Bq"], inputs["Ak"], inputs["Bk"],
        inputs["Av"], inputs["Bv"], inputs["Ao"], inputs["Bo"],
    )
    in_maps = make_core_inputs(x, Wq_eff, Wk_eff, Wv_eff, Wo_eff)
    nc = _get_program(S, C)
    if not FLIP:
        for m in in_maps:
            m.pop("ident", None)  # non-flip program has no ident input
    res = run_bass_kernel_spmd(nc, in_maps, list(range(N_CORES)))
    parts = [res.results[c]["outT_part"].astype(np.float32) for c in range(N_CORES)]
    bo = inputs["bo"]
    out = np.stack(
        [
            (parts[0] + parts[1] + parts[2] + parts[3]).T + bo,
            (parts[4] + parts[5] + parts[6] + parts[7]).T + bo,
        ]
    ).astype(np.float32)
    return out
